# revision 7
# baseline (speedup 1.0000x reference)
"""Differential attention on 8 trn2 NeuronCores.

Sharding: data-parallel over batch (2 groups of 4 cores) x tensor-parallel
over heads (4 heads/core). Each core computes its head-group's qkv
projections, dual softmax attention, and a partial output projection over
its 256 channels, plus the per-token sum-of-squares needed for the RMSNorm.
The host sums the 4 partial projections per batch, applies the RMS scale
(which commutes with the channel contraction) and the bias.

All matmuls run as float32r (full-rate fp32 streaming on the PE).
Layouts are feature-major ([feature, token]) so softmax rowsums ride the
attention@V matmul via a ones-augmented V, avoiding cross-partition
reductions.
"""
import sys

sys.path.insert(0, "/opt/trn_rl_repo")

import numpy as np

import concourse.bass as bass
import concourse.mybir as mybir
import concourse.tile as tile
from concourse import bacc, bass_utils
from concourse.bass_interp import get_hw_module

F32 = mybir.dt.float32
F32R = mybir.dt.float32r
AF = mybir.ActivationFunctionType
OP = mybir.AluOpType
AX = mybir.AxisListType

B, N, DIM = 2, 2048, 1024
H, HD = 16, 64
HPC = 4          # heads per core
CH = HPC * HD    # channels per core (256)
SCALE = HD ** -0.5
EPS = 1e-5
NT = N // 128    # 16 token tiles
QC = N // 512    # 4 query chunks
CT = DIM // 128  # 8 contraction tiles


def r(ap):
    return ap.bitcast(F32R)


def build_program(nc):
    xt = nc.dram_tensor("xt", [DIM, N], F32, kind="ExternalInput").ap()
    wqk = nc.dram_tensor("wqk", [DIM, 8 * 128], F32, kind="ExternalInput").ap()
    wv = nc.dram_tensor("wv", [DIM, CH], F32, kind="ExternalInput").ap()
    wp = nc.dram_tensor("wp", [CH, DIM], F32, kind="ExternalInput").ap()
    lam = nc.dram_tensor("lam", [1, 4 * HD], F32, kind="ExternalInput").ap()
    out = nc.dram_tensor("out", [DIM, N], F32, kind="ExternalOutput").ap()
    ssq = nc.dram_tensor("ssq", [1, N], F32, kind="ExternalOutput").ap()

    with tile.TileContext(nc) as tc:
        with (
            nc.allow_low_precision(reason="float32r matmul operand rounding is intentional"),
            tc.tile_pool(name="persist", bufs=1) as pp,
            tc.tile_pool(name="qkp", bufs=8) as qkpool,
            tc.tile_pool(name="opool", bufs=2) as opool,
        ):
            # ---- constants / lambda ----
            ones_col = pp.tile([1, 64], F32R, tag="ones_col")
            nc.vector.memset(ones_col.bitcast(mybir.dt.uint32)[:], 0x3F800000)
            ones128 = pp.tile([128, 1], F32R, tag="ones128")
            nc.vector.memset(ones128.bitcast(mybir.dt.uint32)[:], 0x3F800000)
            lam_sb = pp.tile([1, 4 * HD], F32, tag="lam")
            nc.sync.dma_start(lam_sb[:], lam[:])
            lprod = pp.tile([1, 2 * HD], F32, tag="lprod")
            nc.vector.tensor_mul(lprod[:, 0:HD], lam_sb[:, 0:HD], lam_sb[:, HD:2 * HD])
            nc.vector.tensor_mul(
                lprod[:, HD:2 * HD], lam_sb[:, 2 * HD:3 * HD], lam_sb[:, 3 * HD:4 * HD]
            )
            lsum = pp.tile([1, 2], F32, tag="lsum")
            nc.vector.reduce_sum(lsum[:, 0:1], lprod[:, 0:HD], axis=AX.X)
            nc.vector.reduce_sum(lsum[:, 1:2], lprod[:, HD:2 * HD], axis=AX.X)
            lexp = pp.tile([1, 2], F32, tag="lexp")
            nc.scalar.activation(lexp[:], lsum[:], AF.Exp)
            negl = pp.tile([1, 1], F32, tag="negl")
            # -lambda_full = exp(sum lq2*lk2) - exp(sum lq1*lk1) - 0.8
            nc.vector.tensor_sub(negl[:], lexp[:, 1:2], lexp[:, 0:1])
            nc.vector.tensor_scalar_add(negl[:], negl[:], -0.8)

            # ---- persistent big tiles ----
            # v augmented with a ones column: [token_part, head, token_tile, hd+1]
            vaug = pp.tile([128, HPC, NT, HD + 1], F32R, tag="vaug")
            nc.vector.memset(vaug[:, :, :, HD:HD + 1].bitcast(mybir.dt.uint32), 0x3F800000)
            qk = [qkpool.tile([128, N], F32R, tag="qk", name=f"qk{i}") for i in range(8)]
            wp_sb = pp.tile([128, 2, DIM], F32R, tag="wp")
            nc.sync.dma_start(wp_sb[:], wp.rearrange("(t p) o -> p t o", p=128).bitcast(F32R))
            o_t = [opool.tile([128, N], F32R, tag="obig", name=f"obig{i}") for i in range(2)]

            # ---- phase A: projections ----
            with (
                tc.tile_pool(name="xa", bufs=1) as xpool,
                tc.tile_pool(name="wa", bufs=2) as wpool,
                tc.tile_pool(name="psA", bufs=3, space="PSUM") as psA,
                tc.tile_pool(name="psV", bufs=2, space="PSUM") as psV,
            ):
                x_sb = xpool.tile([128, CT, N], F32R, tag="x")
                xt_r = xt.rearrange("(t p) n -> p t n", p=128)
                for ct in range(CT):
                    nc.sync.dma_start(x_sb[:, ct, :], xt_r[:, ct, :].bitcast(F32R))
                wv_sb = wpool.tile([128, CT, CH], F32R, tag="wv")
                nc.sync.dma_start(wv_sb[:], wv.rearrange("(t p) f -> p t f", p=128).bitcast(F32R))

                # V in [token, feature] layout, scattered into vaug
                for nt in range(NT):
                    ps = psV.tile([128, CH], F32, tag="psv")
                    for ct in range(CT):
                        nc.tensor.matmul(
                            ps[:],
                            lhsT=r(x_sb[:, ct, nt * 128:(nt + 1) * 128]),
                            rhs=r(wv_sb[:, ct, :]),
                            start=(ct == 0),
                            stop=(ct == CT - 1),
                        )
                    nc.scalar.copy(
                        out=vaug[:, :, nt, 0:HD],
                        in_=ps.rearrange("p (h d) -> p h d", d=HD),
                    )

                # Q/K in [feature, token] layout.
                # f-tile ft<4: [q1(h=ft) 64 | q2(h=ft) 64]; ft>=4: [k1|k2] of h=ft-4
                for ft in range(8):
                    w_sb = wpool.tile([128, CT, 128], F32R, tag="wqk")
                    nc.sync.dma_start(
                        w_sb[:],
                        wqk.rearrange("(t p) f -> p t f", p=128)[
                            :, :, ft * 128:(ft + 1) * 128
                        ].bitcast(F32R),
                    )
                    for qc in range(QC):
                        ps = psA.tile([128, 512], F32, tag="psqk")
                        for ct in range(CT):
                            nc.tensor.matmul(
                                ps[:],
                                lhsT=r(w_sb[:, ct, :]),
                                rhs=r(x_sb[:, ct, qc * 512:(qc + 1) * 512]),
                                start=(ct == 0),
                                stop=(ct == CT - 1),
                            )
                        nc.scalar.copy(out=qk[ft][:, qc * 512:(qc + 1) * 512], in_=ps[:])

            # ---- phase B: attention ----
            with (
                tc.tile_pool(name="slots", bufs=2, space="PSUM") as slots,
                tc.tile_pool(name="po", bufs=2, space="PSUM") as po,
                tc.tile_pool(name="upool", bufs=3) as upool,
                tc.tile_pool(name="cpool", bufs=2) as cpool,
                tc.tile_pool(name="rpool", bufs=2) as rpool,
                tc.tile_pool(name="tpool", bufs=2) as tpool,
            ):
                for h in range(HPC):
                    tq, tk = qk[h], qk[4 + h]
                    for qc in range(QC):
                        o1 = po.tile([HD + 1, 512], F32, tag="oacc")
                        o2 = po.tile([HD + 1, 512], F32, tag="oacc")
                        # 32 blocks: (term, kt); groups of <=3 share a psum slot
                        blocks = [(bi % 2, bi // 2) for bi in range(2 * NT)]
                        for g in range(0, len(blocks), 3):
                            grp = blocks[g:g + 3]
                            nb = len(grp)
                            sl = slots.tile([128, 3 * 512], F32, tag="slot")
                            for j, (term, kt) in enumerate(grp):
                                rb = term * 64
                                nc.tensor.matmul(
                                    sl[:, j * 512:(j + 1) * 512],
                                    lhsT=r(tk[rb:rb + 64, kt * 128:(kt + 1) * 128]),
                                    rhs=r(tq[rb:rb + 64, qc * 512:(qc + 1) * 512]),
                                    start=True,
                                    stop=True,
                                )
                            u = upool.tile([128, 3 * 512], F32R, tag="u")
                            nc.scalar.activation(
                                u[:, 0:nb * 512], sl[:, 0:nb * 512], AF.Exp, scale=SCALE
                            )
                            for j, (term, kt) in enumerate(grp):
                                o = o1 if term == 0 else o2
                                nc.tensor.matmul(
                                    o[:],
                                    lhsT=r(vaug[:, h, kt, :]),
                                    rhs=r(u[:, j * 512:(j + 1) * 512]),
                                    start=(kt == 0),
                                    stop=(kt == NT - 1),
                                )
                        # combine: O_h = o1/r1 - lambda*o2/r2  (per-token scales)
                        o12 = cpool.tile([HD + 1, 1024], F32, tag="o12")
                        nc.vector.tensor_copy(o12[:, 0:512], o1[:])
                        nc.vector.tensor_copy(o12[:, 512:1024], o2[:])
                        rr = rpool.tile([1, 1024], F32R, tag="rr")
                        nc.vector.reciprocal(rr[:, 0:512], o12[HD:HD + 1, 0:512])
                        nc.vector.reciprocal(rr[:, 512:1024], o12[HD:HD + 1, 512:1024])
                        nc.vector.tensor_scalar_mul(
                            rr[:, 512:1024], rr[:, 512:1024], negl[:]
                        )
                        rep = slots.tile([128, 3 * 512], F32, tag="slot")
                        nc.tensor.matmul(
                            rep[0:64, 0:512], lhsT=r(ones_col[:]), rhs=r(rr[:, 0:512]),
                            start=True, stop=True,
                        )
                        nc.tensor.matmul(
                            rep[0:64, 512:1024], lhsT=r(ones_col[:]),
                            rhs=r(rr[:, 512:1024]), start=True, stop=True,
                        )
                        t12 = tpool.tile([HD, 1024], F32, tag="t12")
                        nc.vector.tensor_mul(t12[:], o12[0:HD, :], rep[0:64, 0:1024])
                        nc.vector.tensor_add(
                            o_t[h // 2][
                                (h % 2) * 64:(h % 2) * 64 + 64,
                                qc * 512:(qc + 1) * 512,
                            ],
                            t12[:, 0:512],
                            t12[:, 512:1024],
                        )

            # ---- phase C: sumsq + partial projection ----
            with (
                tc.tile_pool(name="psS", bufs=1, space="PSUM") as psS,
                tc.tile_pool(name="psP", bufs=3, space="PSUM") as psP,
                tc.tile_pool(name="sqpool", bufs=2) as sqpool,
                tc.tile_pool(name="obuf", bufs=3) as obuf,
            ):
                sq = [sqpool.tile([128, N], F32R, tag="sq", name=f"sq{i}") for i in range(2)]
                ssq_ps = psS.tile([1, N], F32, tag="ssqp")
                for t in range(2):
                    nc.vector.tensor_mul(sq[t][:], o_t[t][:], o_t[t][:])
                for c4 in range(QC):
                    for t in range(2):
                        nc.tensor.matmul(
                            ssq_ps[:, c4 * 512:(c4 + 1) * 512],
                            lhsT=r(ones128[:]),
                            rhs=r(sq[t][:, c4 * 512:(c4 + 1) * 512]),
                            start=(t == 0),
                            stop=(t == 1),
                        )
                ssq_sb = sqpool.tile([1, N], F32, tag="ssqs")
                nc.vector.tensor_copy(ssq_sb[:], ssq_ps[:])
                nc.sync.dma_start(ssq[:], ssq_sb[:])

                for ot in range(8):
                    for nch in range(QC):
                        ps = psP.tile([128, 512], F32, tag="psp")
                        for t in range(2):
                            nc.tensor.matmul(
                                ps[:],
                                lhsT=r(wp_sb[:, t, ot * 128:(ot + 1) * 128]),
                                rhs=r(o_t[t][:, nch * 512:(nch + 1) * 512]),
                                start=(t == 0),
                                stop=(t == 1),
                            )
                        ob = obuf.tile([128, 512], F32, tag="ob")
                        nc.vector.tensor_copy(ob[:], ps[:])
                        nc.sync.dma_start(
                            out[ot * 128:(ot + 1) * 128, nch * 512:(nch + 1) * 512],
                            ob[:],
                        )
    return nc


_CACHE = {}


def get_nc():
    if "nc" not in _CACHE:
        nc = bacc.Bacc(
            "TRN2", target_bir_lowering=False, debug=False, enable_asserts=False
        )
        build_program(nc)
        nc.compile()
        nc.m = get_hw_module(nc.m)
        _CACHE["nc"] = nc
    return _CACHE["nc"]


def make_in_maps(x, qkv_w, proj_w, lambda_q1, lambda_k1, lambda_q2, lambda_k2):
    x = np.asarray(x, np.float32)
    qkv_w = np.asarray(qkv_w, np.float32)
    proj_w = np.asarray(proj_w, np.float32)
    lamv = np.concatenate(
        [np.asarray(a, np.float32) for a in (lambda_q1, lambda_k1, lambda_q2, lambda_k2)]
    )[None, :]
    in_maps = []
    for core in range(8):
        b, hg = core // 4, core % 4
        h0 = hg * HPC
        rows = []
        for h in range(h0, h0 + HPC):
            rows.append(qkv_w[0 * DIM + h * HD:0 * DIM + (h + 1) * HD])
            rows.append(qkv_w[1 * DIM + h * HD:1 * DIM + (h + 1) * HD])
        for h in range(h0, h0 + HPC):
            rows.append(qkv_w[2 * DIM + h * HD:2 * DIM + (h + 1) * HD])
            rows.append(qkv_w[3 * DIM + h * HD:3 * DIM + (h + 1) * HD])
        wqk_np = np.ascontiguousarray(np.concatenate(rows, 0).T)
        wv_np = np.ascontiguousarray(
            np.concatenate(
                [qkv_w[4 * DIM + h * HD:4 * DIM + (h + 1) * HD] for h in range(h0, h0 + HPC)],
                0,
            ).T
        )
        wp_np = np.ascontiguousarray(proj_w[:, h0 * HD:(h0 + HPC) * HD].T)
        in_maps.append(
            {
                "xt": np.ascontiguousarray(x[b].T),
                "wqk": wqk_np,
                "wv": wv_np,
                "wp": wp_np,
                "lam": np.ascontiguousarray(lamv),
            }
        )
    return in_maps


def combine(results, proj_b):
    proj_b = np.asarray(proj_b, np.float32)
    y = np.empty((B, N, DIM), np.float32)
    for b in range(B):
        acc = np.zeros((DIM, N), np.float64)
        sq = np.zeros(N, np.float64)
        for g in range(4):
            rr = results[b * 4 + g]
            acc += rr["out"].astype(np.float64)
            sq += rr["ssq"][0].astype(np.float64)
        s = 0.2 / np.sqrt(sq / DIM + EPS)
        y[b] = (acc.T * s[:, None] + proj_b).astype(np.float32)
    return y


def kernel(x, qkv_w, proj_w, proj_b, lambda_q1, lambda_k1, lambda_q2, lambda_k2):
    nc = get_nc()
    in_maps = make_in_maps(
        x, qkv_w, proj_w, lambda_q1, lambda_k1, lambda_q2, lambda_k2
    )
    res = bass_utils.run_bass_kernel_spmd(nc, in_maps, core_ids=list(range(8)))
    return combine(res.results, proj_b)


# revision 10
# speedup vs baseline: 1.2083x; 1.2083x over previous
"""Differential attention on 8 trn2 NeuronCores.

Sharding: data-parallel over batch (2 groups of 4 cores) x tensor-parallel
over heads (4 heads/core). Each core computes its head-group's qkv
projections, dual softmax attention, and a partial output projection over
its 256 channels, plus the per-token sum-of-squares needed for the RMSNorm.
The host sums the 4 partial projections per batch, applies the RMS scale
(which commutes with the channel contraction) and the bias.

All matmuls run as float32r (full-rate fp32 streaming on the PE).
Layouts are feature-major ([feature, token]) so softmax rowsums ride the
attention@V matmul via a ones-augmented V, avoiding cross-partition
reductions.
"""
import sys

sys.path.insert(0, "/opt/trn_rl_repo")

import numpy as np

import concourse.bass as bass
import concourse.mybir as mybir
import concourse.tile as tile
from concourse import bacc, bass_utils
from concourse.bass_interp import get_hw_module

F32 = mybir.dt.float32
F32R = mybir.dt.float32r
AF = mybir.ActivationFunctionType
OP = mybir.AluOpType
AX = mybir.AxisListType

B, N, DIM = 2, 2048, 1024
H, HD = 16, 64
HPC = 4          # heads per core
CH = HPC * HD    # channels per core (256)
SCALE = HD ** -0.5
EPS = 1e-5
NT = N // 128    # 16 token tiles
QC = N // 512    # 4 query chunks
CT = DIM // 128  # 8 contraction tiles


def r(ap):
    return ap.bitcast(F32R)


def build_program(nc):
    xt = nc.dram_tensor("xt", [DIM, N], F32, kind="ExternalInput").ap()
    wqk = nc.dram_tensor("wqk", [DIM, 8 * 128], F32, kind="ExternalInput").ap()
    wv = nc.dram_tensor("wv", [DIM, CH], F32, kind="ExternalInput").ap()
    wp = nc.dram_tensor("wp", [CH, DIM], F32, kind="ExternalInput").ap()
    lam = nc.dram_tensor("lam", [1, 4 * HD], F32, kind="ExternalInput").ap()
    out = nc.dram_tensor("out", [DIM, N], F32, kind="ExternalOutput").ap()
    ssq = nc.dram_tensor("ssq", [1, N], F32, kind="ExternalOutput").ap()

    with tile.TileContext(nc) as tc:
        with (
            nc.allow_low_precision(reason="float32r matmul operand rounding is intentional"),
            tc.tile_pool(name="persist", bufs=1) as pp,
            tc.tile_pool(name="qkp", bufs=8) as qkpool,
            tc.tile_pool(name="opool", bufs=2) as opool,
        ):
            # ---- constants / lambda ----
            ones_col = pp.tile([1, 64], F32R, tag="ones_col")
            nc.vector.memset(ones_col.bitcast(mybir.dt.uint32)[:], 0x3F800000)
            ones128 = pp.tile([128, 1], F32R, tag="ones128")
            nc.vector.memset(ones128.bitcast(mybir.dt.uint32)[:], 0x3F800000)
            lam_sb = pp.tile([1, 4 * HD], F32, tag="lam")
            nc.sync.dma_start(lam_sb[:], lam[:])
            lprod = pp.tile([1, 2 * HD], F32, tag="lprod")
            nc.vector.tensor_mul(lprod[:, 0:HD], lam_sb[:, 0:HD], lam_sb[:, HD:2 * HD])
            nc.vector.tensor_mul(
                lprod[:, HD:2 * HD], lam_sb[:, 2 * HD:3 * HD], lam_sb[:, 3 * HD:4 * HD]
            )
            lsum = pp.tile([1, 2], F32, tag="lsum")
            nc.vector.reduce_sum(lsum[:, 0:1], lprod[:, 0:HD], axis=AX.X)
            nc.vector.reduce_sum(lsum[:, 1:2], lprod[:, HD:2 * HD], axis=AX.X)
            lexp = pp.tile([1, 2], F32, tag="lexp")
            nc.scalar.activation(lexp[:], lsum[:], AF.Exp)
            negl = pp.tile([1, 1], F32, tag="negl")
            # -lambda_full = exp(sum lq2*lk2) - exp(sum lq1*lk1) - 0.8
            nc.vector.tensor_sub(negl[:], lexp[:, 1:2], lexp[:, 0:1])
            nc.vector.tensor_scalar_add(negl[:], negl[:], -0.8)

            # ---- persistent big tiles ----
            # v augmented with a ones column: [token_part, head, token_tile, hd+1]
            vaug = pp.tile([128, HPC, NT, HD + 1], F32R, tag="vaug")
            nc.vector.memset(vaug[:, :, :, HD:HD + 1].bitcast(mybir.dt.uint32), 0x3F800000)
            qk = [qkpool.tile([128, N], F32R, tag="qk", name=f"qk{i}") for i in range(8)]
            wp_sb = pp.tile([128, 2, DIM], F32R, tag="wp")
            nc.sync.dma_start(wp_sb[:], wp.rearrange("(t p) o -> p t o", p=128).bitcast(F32R))
            o_t = [opool.tile([128, N], F32R, tag="obig", name=f"obig{i}") for i in range(2)]

            # ---- phase A: projections ----
            with (
                tc.tile_pool(name="xa", bufs=1) as xpool,
                tc.tile_pool(name="wa", bufs=2) as wpool,
                tc.tile_pool(name="psA", bufs=3, space="PSUM") as psA,
                tc.tile_pool(name="psV", bufs=2, space="PSUM") as psV,
            ):
                x_sb = xpool.tile([128, CT, N], F32R, tag="x")
                xt_r = xt.rearrange("(t p) n -> p t n", p=128)
                for ch in range(QC):
                    nc.sync.dma_start(
                        x_sb[:, :, ch * 512:(ch + 1) * 512],
                        xt_r[:, :, ch * 512:(ch + 1) * 512].bitcast(F32R),
                    )
                wv_sb = wpool.tile([128, CT, CH], F32R, tag="wv")
                nc.sync.dma_start(wv_sb[:], wv.rearrange("(t p) f -> p t f", p=128).bitcast(F32R))

                # V in [token, feature] layout, scattered into vaug
                for nt in range(NT):
                    ps = psV.tile([128, CH], F32, tag="psv")
                    for ct in range(CT):
                        nc.tensor.matmul(
                            ps[:],
                            lhsT=r(x_sb[:, ct, nt * 128:(nt + 1) * 128]),
                            rhs=r(wv_sb[:, ct, :]),
                            start=(ct == 0),
                            stop=(ct == CT - 1),
                        )
                    nc.scalar.copy(
                        out=vaug[:, :, nt, 0:HD],
                        in_=ps.rearrange("p (h d) -> p h d", d=HD),
                    )

                # Q/K in [feature, token] layout.
                # f-tile ft<4: [q1(h=ft) 64 | q2(h=ft) 64]; ft>=4: [k1|k2] of h=ft-4
                for ft in range(8):
                    w_sb = wpool.tile([128, CT, 128], F32R, tag="wqk")
                    nc.sync.dma_start(
                        w_sb[:],
                        wqk.rearrange("(t p) f -> p t f", p=128)[
                            :, :, ft * 128:(ft + 1) * 128
                        ].bitcast(F32R),
                    )
                    for qc in range(QC):
                        ps = psA.tile([128, 512], F32, tag="psqk")
                        for ct in range(CT):
                            nc.tensor.matmul(
                                ps[:],
                                lhsT=r(w_sb[:, ct, :]),
                                rhs=r(x_sb[:, ct, qc * 512:(qc + 1) * 512]),
                                start=(ct == 0),
                                stop=(ct == CT - 1),
                            )
                        nc.scalar.copy(out=qk[ft][:, qc * 512:(qc + 1) * 512], in_=ps[:])

            # ---- phase B: attention ----
            with (
                tc.tile_pool(name="slots", bufs=2, space="PSUM") as slots,
                tc.tile_pool(name="po", bufs=2, space="PSUM") as po,
                tc.tile_pool(name="upool", bufs=2) as upool,
                tc.tile_pool(name="cpool", bufs=8) as cpool,
                tc.tile_pool(name="rpool", bufs=8) as rpool,
                tc.tile_pool(name="tpool", bufs=2) as tpool,
            ):
                pending = []

                def flush_pending():
                    # combine part 2: replicate per-token scales across the
                    # 64 hd-rows via K=1 outer-product matmuls, then
                    # scale + add. Emitted one head late so the PE queue
                    # never stalls on the DVE reciprocals (keeps HAM warm).
                    for fh, fqc, o12, rr in pending:
                        rep = slots.tile([128, 3 * 512], F32, tag="slot",
                                         name=f"rep_{fh}_{fqc}")
                        nc.tensor.matmul(
                            rep[0:64, 0:512], lhsT=r(ones_col[:]),
                            rhs=r(rr[:, 0:512]), start=True, stop=True,
                        )
                        nc.tensor.matmul(
                            rep[0:64, 512:1024], lhsT=r(ones_col[:]),
                            rhs=r(rr[:, 512:1024]), start=True, stop=True,
                        )
                        t12 = tpool.tile([HD, 1024], F32, tag="t12")
                        nc.vector.tensor_mul(
                            t12[:], o12[0:HD, :], rep[0:64, 0:1024]
                        )
                        nc.vector.tensor_add(
                            o_t[fh // 2][
                                (fh % 2) * 64:(fh % 2) * 64 + 64,
                                fqc * 512:(fqc + 1) * 512,
                            ],
                            t12[:, 0:512],
                            t12[:, 512:1024],
                        )
                    pending.clear()

                for h in range(HPC):
                    tq, tk = qk[h], qk[4 + h]
                    for qc in range(QC):
                        if qc == 1 and pending:
                            flush_pending()
                        o1 = po.tile([HD + 1, 512], F32, tag="oacc")
                        o2 = po.tile([HD + 1, 512], F32, tag="oacc")
                        # 32 blocks: (term, kt); groups of <=3 share a psum slot
                        blocks = [(bi % 2, bi // 2) for bi in range(2 * NT)]
                        for g in range(0, len(blocks), 3):
                            grp = blocks[g:g + 3]
                            nb = len(grp)
                            sl = slots.tile([128, 3 * 512], F32, tag="slot")
                            for j, (term, kt) in enumerate(grp):
                                rb = term * 64
                                nc.tensor.matmul(
                                    sl[:, j * 512:(j + 1) * 512],
                                    lhsT=r(tk[rb:rb + 64, kt * 128:(kt + 1) * 128]),
                                    rhs=r(tq[rb:rb + 64, qc * 512:(qc + 1) * 512]),
                                    start=True,
                                    stop=True,
                                )
                            u = upool.tile([128, 3 * 512], F32R, tag="u")
                            nc.scalar.activation(
                                u[:, 0:nb * 512], sl[:, 0:nb * 512], AF.Exp, scale=SCALE
                            )
                            for j, (term, kt) in enumerate(grp):
                                o = o1 if term == 0 else o2
                                nc.tensor.matmul(
                                    o[:],
                                    lhsT=r(vaug[:, h, kt, :]),
                                    rhs=r(u[:, j * 512:(j + 1) * 512]),
                                    start=(kt == 0),
                                    stop=(kt == NT - 1),
                                )
                        # combine part 1 (off the PE critical path):
                        # copy o accumulators out and build [1/r1 | -lam/r2]
                        o12 = cpool.tile([HD + 1, 1024], F32, tag="o12",
                                         name=f"o12_{h}_{qc}")
                        nc.vector.tensor_copy(o12[:, 0:512], o1[:])
                        nc.vector.tensor_copy(o12[:, 512:1024], o2[:])
                        rr = rpool.tile([1, 1024], F32R, tag="rr",
                                        name=f"rr_{h}_{qc}")
                        nc.vector.reciprocal(rr[:, 0:512], o12[HD:HD + 1, 0:512])
                        nc.vector.reciprocal(rr[:, 512:1024], o12[HD:HD + 1, 512:1024])
                        nc.vector.tensor_scalar_mul(
                            rr[:, 512:1024], rr[:, 512:1024], negl[:]
                        )
                        pending.append((h, qc, o12, rr))
                flush_pending()

            # ---- phase C: sumsq + partial projection ----
            with (
                tc.tile_pool(name="psS", bufs=1, space="PSUM") as psS,
                tc.tile_pool(name="psP", bufs=3, space="PSUM") as psP,
                tc.tile_pool(name="sqpool", bufs=2) as sqpool,
                tc.tile_pool(name="obuf", bufs=3) as obuf,
            ):
                sq = [sqpool.tile([128, N], F32R, tag="sq", name=f"sq{i}") for i in range(2)]
                ssq_ps = psS.tile([1, N], F32, tag="ssqp")
                for t in range(2):
                    nc.vector.tensor_mul(sq[t][:], o_t[t][:], o_t[t][:])
                for c4 in range(QC):
                    for t in range(2):
                        nc.tensor.matmul(
                            ssq_ps[:, c4 * 512:(c4 + 1) * 512],
                            lhsT=r(ones128[:]),
                            rhs=r(sq[t][:, c4 * 512:(c4 + 1) * 512]),
                            start=(t == 0),
                            stop=(t == 1),
                        )
                ssq_sb = sqpool.tile([1, N], F32, tag="ssqs")
                nc.vector.tensor_copy(ssq_sb[:], ssq_ps[:])
                nc.sync.dma_start(ssq[:], ssq_sb[:])

                for ot in range(8):
                    for nch in range(QC):
                        ps = psP.tile([128, 512], F32, tag="psp")
                        for t in range(2):
                            nc.tensor.matmul(
                                ps[:],
                                lhsT=r(wp_sb[:, t, ot * 128:(ot + 1) * 128]),
                                rhs=r(o_t[t][:, nch * 512:(nch + 1) * 512]),
                                start=(t == 0),
                                stop=(t == 1),
                            )
                        ob = obuf.tile([128, 512], F32, tag="ob")
                        nc.vector.tensor_copy(ob[:], ps[:])
                        nc.sync.dma_start(
                            out[ot * 128:(ot + 1) * 128, nch * 512:(nch + 1) * 512],
                            ob[:],
                        )
    return nc


_CACHE = {}


def get_nc():
    if "nc" not in _CACHE:
        nc = bacc.Bacc(
            "TRN2", target_bir_lowering=False, debug=False, enable_asserts=False
        )
        build_program(nc)
        nc.compile()
        nc.m = get_hw_module(nc.m)
        _CACHE["nc"] = nc
    return _CACHE["nc"]


def make_in_maps(x, qkv_w, proj_w, lambda_q1, lambda_k1, lambda_q2, lambda_k2):
    x = np.asarray(x, np.float32)
    qkv_w = np.asarray(qkv_w, np.float32)
    proj_w = np.asarray(proj_w, np.float32)
    lamv = np.concatenate(
        [np.asarray(a, np.float32) for a in (lambda_q1, lambda_k1, lambda_q2, lambda_k2)]
    )[None, :]
    in_maps = []
    for core in range(8):
        b, hg = core // 4, core % 4
        h0 = hg * HPC
        rows = []
        for h in range(h0, h0 + HPC):
            rows.append(qkv_w[0 * DIM + h * HD:0 * DIM + (h + 1) * HD])
            rows.append(qkv_w[1 * DIM + h * HD:1 * DIM + (h + 1) * HD])
        for h in range(h0, h0 + HPC):
            rows.append(qkv_w[2 * DIM + h * HD:2 * DIM + (h + 1) * HD])
            rows.append(qkv_w[3 * DIM + h * HD:3 * DIM + (h + 1) * HD])
        wqk_np = np.ascontiguousarray(np.concatenate(rows, 0).T)
        wv_np = np.ascontiguousarray(
            np.concatenate(
                [qkv_w[4 * DIM + h * HD:4 * DIM + (h + 1) * HD] for h in range(h0, h0 + HPC)],
                0,
            ).T
        )
        wp_np = np.ascontiguousarray(proj_w[:, h0 * HD:(h0 + HPC) * HD].T)
        in_maps.append(
            {
                "xt": np.ascontiguousarray(x[b].T),
                "wqk": wqk_np,
                "wv": wv_np,
                "wp": wp_np,
                "lam": np.ascontiguousarray(lamv),
            }
        )
    return in_maps


def combine(results, proj_b):
    proj_b = np.asarray(proj_b, np.float32)
    y = np.empty((B, N, DIM), np.float32)
    for b in range(B):
        acc = np.zeros((DIM, N), np.float64)
        sq = np.zeros(N, np.float64)
        for g in range(4):
            rr = results[b * 4 + g]
            acc += rr["out"].astype(np.float64)
            sq += rr["ssq"][0].astype(np.float64)
        s = 0.2 / np.sqrt(sq / DIM + EPS)
        y[b] = (acc.T * s[:, None] + proj_b).astype(np.float32)
    return y


def kernel(x, qkv_w, proj_w, proj_b, lambda_q1, lambda_k1, lambda_q2, lambda_k2):
    nc = get_nc()
    in_maps = make_in_maps(
        x, qkv_w, proj_w, lambda_q1, lambda_k1, lambda_q2, lambda_k2
    )
    res = bass_utils.run_bass_kernel_spmd(nc, in_maps, core_ids=list(range(8)))
    return combine(res.results, proj_b)
